# revision 9
# baseline (speedup 1.0000x reference)
"""DMoN head kernel for 8x Trainium2 NeuronCores (Bass/Tile, SPMD).

Strategy
--------
Only trace(S^T A S) is needed from the big adjacency contraction, and the
adjacency depends only on edge_index (an input), so the host builds the dense
0/1 adjacency (exact in fp8) and column-shards it over 8 cores.

Rotation trick: core c receives every row-indexed input rotated by c*BLK so
the single SPMD program uses only static slices -- each core's "first BLK
rows" are its own shard. Each core redundantly computes h = selu(emb@W^T+b),
logits, s = softmax (cheap), then contracts its A column-block:
  T = [s_hi|s_lo]^T @ A_blk   (bf16 hi/lo split of s packed as one 128-wide
                               stationary, fp8 A moving -> fp32-exact)
and reduces diag_k = sum_local s[local,k]*T[k,local] on-device.  The K-sized
side terms (ss = S^T S, S^T deg, cluster_size, M^T S) ride as ONE extra
matmul per contraction tile off the same stationary, into one PSUM bank.
Host sums the 8 per-core partials (the "KxK all-reduce" of the sharding
hint) and assembles the scalar losses.

TRI mode halves A traffic: keep each undirected pair once (wrap-distance
rule, core-independent under rotation), pack only live 512-wide blocks, and
let the host add the (tiny) self-loop term; trace = 2*sum(diag) + self_term.
"""

import os
import sys

import numpy as np

for _p in ("/opt/trn_rl_repo", "/root/.axon_site/_ro/trn_rl_repo"):
    if os.path.isdir(_p) and _p not in sys.path:
        sys.path.insert(0, _p)

import ml_dtypes  # noqa: E402

BF16 = ml_dtypes.bfloat16
FP8 = ml_dtypes.float8_e4m3fn

LAM = 1.0507009873554805  # selu lambda
ALPHA = 1.6732632423543772  # selu alpha
NCORES = 8
NTYPES = 17
HLW = 128 + 2 + NTYPES  # hilo(128) | deg | ones | one-hot(17)
SMALLW = HLW
TRI = os.environ.get("DMON_TRI", "1") == "1"

_nc_cache = {}


# ---------------------------------------------------------------------------
# static liveness for TRI mode: block of A_half rows r in [128g,128g+128),
# cols l in [512h, 512h+512) is live iff some (l-r) mod n lies in [1, n/2].
def _tri_live(ntot, g, h, hw):
    lo = (hw * h - 128 * g - 127) % ntot
    width = 128 + hw - 1
    for d in range(lo, lo + width + 1):
        dm = d % ntot
        if 1 <= dm <= ntot // 2:
            return True
    return False


def _live_blocks(ntot, blk):
    g_tiles = ntot // 128
    hw = min(blk, 512)
    nh = max(1, blk // hw)
    out = []
    for g in range(g_tiles):
        for h in range(nh):
            if not TRI or _tri_live(ntot, g, h, hw):
                out.append((g, h))
    return out


# ---------------------------------------------------------------------------
def _legalize_waits(nc, mybir):
    """This sandbox's walrus encodes at most ONE sync wait per instruction;
    Tile emits up to ~3. Hoist extra waits into standalone same-engine
    EventSemaphore (wait-only) instructions placed just before the owner."""
    n = 0
    for f in nc.m.functions:
        for blk in f.blocks:
            out = []
            for inst in blk.instructions:
                si = inst.sync_info
                if (si is not None and len(si.on_wait) > 1
                        and inst.opcode != "EventSemaphore"
                        and inst.engine != mybir.EngineType.Unassigned):
                    waits = list(si.on_wait)
                    for w in waits[:-1]:
                        n += 1
                        ev = mybir.InstEventSemaphore(
                            name=f"legw-{n}", engine=inst.engine,
                            sync_info=mybir.SyncInfo(on_wait=[w], on_update=[]))
                        nc.register_instruction(ev)
                        out.append(ev)
                    inst.sync_info = mybir.SyncInfo(
                        on_wait=[waits[-1]], on_update=list(si.on_update))
                out.append(inst)
            blk.instructions[:] = out
    return n


def build_bass(ntot, blk, nd=2):
    """Build the SPMD Bass program (identical on all cores).

    nd: number of 128-wide contraction tiles for the encoder matmul
        (2 for D=256; 3 when a nonzero b_enc is folded in as an extra
        ones-column contraction tile)."""
    import concourse.bass as bass
    import concourse.tile as tile
    from concourse import mybir

    f32 = mybir.dt.float32
    bf16 = mybir.dt.bfloat16
    f8e4 = mybir.dt.float8e4
    AF = mybir.ActivationFunctionType
    OP = mybir.AluOpType

    G = ntot // 128          # contraction tiles over rows
    NP = ntot // 512         # h/softmax panels (4 row-tiles each)
    HW_ = min(blk, 512)      # A column-block width
    NH = max(1, blk // HW_)  # A column blocks per core
    NOUT = blk // 128        # output row-tiles per core
    live = _live_blocks(ntot, blk)
    ncols_a = len(live) * HW_
    # A DMA chunking (tile-major packed [128, ncols_a] fp8)
    ACH = None
    for cand in (8192, 6144, 4096, 3072, 2048, 1536, 1024, 512, HW_):
        if cand <= ncols_a and ncols_a % cand == 0:
            ACH = cand
            break
    ECH = 1024 if ntot % 1024 == 0 else 512  # embT col chunk

    nc = bass.Bass()

    # register ln(alpha) as a const AP (activation float biases need one)
    lnALPHA = float(np.log(ALPHA))
    _cl = nc.alloc_sbuf_tensor("const-lnalpha", [128, 1], f32)
    nc.gpsimd.memset(_cl.ap(), lnALPHA)
    nc.const_aps.aps[(f32, lnALPHA)] = _cl.ap()
    nc.all_engine_barrier()

    # inputs (per core, pre-rotated+packed by host)
    A_d = nc.declare_dram_parameter("a_pk", [128, ncols_a], f8e4, isOutput=False)
    et_hi = nc.declare_dram_parameter("et_hi", [128, nd, ntot], bf16, isOutput=False)
    et_lo = nc.declare_dram_parameter("et_lo", [128, nd, ntot], bf16, isOutput=False)
    wt_hi = nc.declare_dram_parameter("wt_hi", [128, nd, 256], bf16, isOutput=False)
    wt_lo = nc.declare_dram_parameter("wt_lo", [128, nd, 256], bf16, isOutput=False)
    cen_d = nc.declare_dram_parameter("cen", [128, 2, 64], f32, isOutput=False)
    dgm_d = nc.declare_dram_parameter("dgm", [128, G, 2 + NTYPES], bf16,
                                      isOutput=False)
    id_d = nc.declare_dram_parameter("ident", [128, 128], f32, isOutput=False)

    # outputs
    lg_o = nc.declare_dram_parameter("logits_sh", [blk, 64], f32, isOutput=True)
    s_o = nc.declare_dram_parameter("s_sh", [blk, 64], f32, isOutput=True)
    dg_o = nc.declare_dram_parameter("diag", [64, NH], f32, isOutput=True)
    sm_o = nc.declare_dram_parameter("small", [128, SMALLW], f32, isOutput=True)

    with tile.TileContext(nc) as tc:
        with (
            tc.tile_pool(name="const", bufs=1) as cpool,
            tc.tile_pool(name="hT", bufs=1) as hpool,
            tc.tile_pool(name="achunk", bufs=3) as apool,
            tc.tile_pool(name="embc", bufs=2) as epool,
            tc.tile_pool(name="work", bufs=4) as wpool,
            tc.tile_pool(name="panel", bufs=3) as ppool,
            tc.tile_pool(name="tail", bufs=8) as tpool,
            tc.tile_pool(name="hps", bufs=2, space=bass.MemorySpace.PSUM) as hps,
            tc.tile_pool(name="lps", bufs=2, space=bass.MemorySpace.PSUM) as lps,
            tc.tile_pool(name="accps", bufs=1, space=bass.MemorySpace.PSUM) as accps,
        ):
            # hot-path constants first (gates the first matmul)
            wthi = cpool.tile([128, nd, 256], bf16, tag="wthi")
            wtlo = cpool.tile([128, nd, 256], bf16, tag="wtlo")
            nc.sync.dma_start(wthi[:], wt_hi[:])
            nc.sync.dma_start(wtlo[:], wt_lo[:])
            # first embT chunk
            et_hi_sb = epool.tile([128, nd, ECH], bf16, tag="ehi")
            et_lo_sb = epool.tile([128, nd, ECH], bf16, tag="elo")
            nc.sync.dma_start(et_hi_sb[:], et_hi[:, :, 0:ECH])
            nc.sync.dma_start(et_lo_sb[:], et_lo[:, :, 0:ECH])
            # remaining residents
            cen = cpool.tile([128, 2, 64], f32, tag="cen")
            dgm = cpool.tile([128, G, 2 + NTYPES], bf16, tag="dgm")
            idn = cpool.tile([128, 128], f32, tag="idn")
            nc.sync.dma_start(cen[:], cen_d[:])
            nc.sync.dma_start(dgm[:], dgm_d[:])
            nc.sync.dma_start(idn[:], id_d[:])

            sth = cpool.tile([64, blk], f32, tag="sth")
            diag_sb = cpool.tile([64, NH], f32, tag="diag")
            # persistent hT (f32) and manual-slot hilo+consts tile
            hT = [hpool.tile([128, ntot], f32, tag=f"hT{t}", name=f"hT{t}")
                  for t in range(2)]
            hl = cpool.tile([128, 8, HLW], bf16, tag="hl")

            # persistent PSUM accumulators
            TA = [accps.tile([128, HW_], f32, tag=f"TA{h}", name=f"TA{h}")
                  for h in range(NH)]
            smp = accps.tile([128, SMALLW], f32, tag="smp")

            a_sb = None
            a_next = 0  # next live-block index not yet covered by DMA
            li = 0      # live-block cursor
            last_g = {}
            first_g = {}
            for (g_, h_) in live:
                last_g[h_] = max(last_g.get(h_, -1), g_)
                first_g.setdefault(h_, g_)
            live_set = set(live)

            lnALPHA = float(np.log(ALPHA))

            for p in range(NP):
                if p > 0 and (p * 512) % ECH == 0:
                    c0 = p * 512
                    et_hi_sb = epool.tile([128, nd, ECH], bf16, tag="ehi")
                    et_lo_sb = epool.tile([128, nd, ECH], bf16, tag="elo")
                    nc.sync.dma_start(et_hi_sb[:], et_hi[:, :, c0:c0 + ECH])
                    nc.sync.dma_start(et_lo_sb[:], et_lo[:, :, c0:c0 + ECH])
                co = (p * 512) % ECH  # col offset within chunk
                sl = (p % 2) * 4      # hl slot range start for this panel

                # ---- h panel: hT[t1][:, p*512:(p+1)*512] = selu(x)/lam ----
                for t1 in range(2):
                    ph = hps.tile([128, 512], f32, tag="hps")
                    nmm = 0
                    for t0 in range(nd):
                        for (w, e) in ((wthi, et_hi_sb), (wtlo, et_hi_sb),
                                       (wthi, et_lo_sb)):
                            nc.tensor.matmul(
                                ph[:],
                                w[:, t0, t1 * 128:(t1 + 1) * 128],
                                e[:, t0, co:co + 512],
                                start=(nmm == 0), stop=(nmm == 3 * nd - 1),
                            )
                            nmm += 1
                    # selu(x)/lam = relu(x) + (min(alpha*e^x, alpha) - alpha)
                    ex = wpool.tile([128, 512], f32, tag="ex")
                    nc.scalar.activation(ex[:], ph[:], AF.Exp, bias=lnALPHA,
                                         scale=1.0)
                    rl = wpool.tile([128, 512], f32, tag="rl")
                    nc.scalar.activation(rl[:], ph[:], AF.Relu)
                    mn = wpool.tile([128, 512], f32, tag="mn")
                    nc.gpsimd.tensor_scalar(mn[:], ex[:], ALPHA, -ALPHA,
                                            OP.min, OP.add)
                    nc.vector.tensor_add(hT[t1][:, p * 512:(p + 1) * 512],
                                         mn[:], rl[:])

                # per-g side constants into hl slot tails (one op per panel)
                nc.gpsimd.tensor_copy(hl[:, sl:sl + 4, 128:HLW],
                                      dgm[:, p * 4:p * 4 + 4, :])

                # ---- logits for the 4 row-tiles of this panel ----
                lgp = ppool.tile([128, 4, 64], f32, tag="lgp")
                for gi in range(4):
                    g = p * 4 + gi
                    lp = lps.tile([128, 64], f32, tag="lg")
                    nc.tensor.matmul(lp[:],
                                     hT[0][:, g * 128:(g + 1) * 128],
                                     cen[:, 0, :], start=True, stop=False)
                    nc.tensor.matmul(lp[:],
                                     hT[1][:, g * 128:(g + 1) * 128],
                                     cen[:, 1, :], start=False, stop=True)
                    nc.scalar.activation(lgp[:, gi, :], lp[:], AF.Copy,
                                         bias=0.0, scale=LAM / 16.0)

                # ---- panel softmax (no max-sub; |logits| is O(1)) ----
                ep = ppool.tile([128, 4, 64], f32, tag="ep")
                nc.scalar.activation(ep[:], lgp[:], AF.Exp)
                rs = ppool.tile([128, 4], f32, tag="rs")
                nc.vector.tensor_reduce(rs[:], ep[:], mybir.AxisListType.X,
                                        OP.add)
                rc = ppool.tile([128, 4, 1], f32, tag="rc")
                nc.vector.reciprocal(rc[:, :, 0], rs[:])
                sp = ppool.tile([128, 4, 64], f32, tag="sp")
                nc.vector.tensor_tensor(sp[:], ep[:],
                                        rc[:].broadcast_to([128, 4, 64]),
                                        op=OP.mult)
                # bf16 hi/lo split into hl slots
                nc.gpsimd.tensor_copy(hl[:, sl:sl + 4, 0:64], sp[:])
                nc.vector.tensor_sub(hl[:, sl:sl + 4, 64:128], sp[:],
                                     hl[:, sl:sl + 4, 0:64])

                # ---- per row-tile: outputs, transpose, A-chain ----
                for gi in range(4):
                    g = p * 4 + gi
                    s = sl + gi
                    if g < NOUT:
                        nc.sync.dma_start(lg_o[g * 128:(g + 1) * 128, :],
                                          lgp[:, gi, :])
                        nc.sync.dma_start(s_o[g * 128:(g + 1) * 128, :],
                                          sp[:, gi, :])
                        tp = lps.tile([64, 128], f32, tag="lg")
                        nc.tensor.transpose(tp[:], sp[:, gi, :], idn[:])
                        nc.vector.tensor_copy(sth[:, g * 128:(g + 1) * 128],
                                              tp[:])
                    for h in range(NH):
                        if (g, h) not in live_set:
                            continue
                        if li >= a_next:  # need next A chunk
                            a_sb = apool.tile([128, ACH], f8e4, tag="ach")
                            nc.sync.dma_start(
                                a_sb[:],
                                A_d[:, a_next * HW_:a_next * HW_ + ACH])
                            a_next += ACH // HW_
                        base = (li % (ACH // HW_)) * HW_
                        nc.tensor.matmul(TA[h][:], hl[:, s, 0:128],
                                         a_sb[:, base:base + HW_],
                                         start=(g == first_g[h]),
                                         stop=(g == last_g[h]),
                                         skip_group_check=True)
                        li += 1
                    # all K-sized side terms in one matmul
                    nc.tensor.matmul(smp[:], hl[:, s, 0:128], hl[:, s, :],
                                     start=(g == 0), stop=(g == G - 1),
                                     skip_group_check=True)

            # ---- tail: T = hi+lo, diag_k = sum_l T*s^T, copy small out ----
            for h in range(NH):
                lo_sb = tpool.tile([64, HW_], f32, tag="tl")
                nc.vector.tensor_copy(lo_sb[:], TA[h][64:128, :])
                t_sb = tpool.tile([64, HW_], f32, tag="tl")
                nc.vector.tensor_add(t_sb[:], TA[h][0:64, :], lo_sb[:])
                prod = tpool.tile([64, HW_], f32, tag="tl")
                nc.vector.tensor_mul(prod[:], t_sb[:],
                                     sth[:, h * HW_:(h + 1) * HW_])
                nc.vector.reduce_sum(diag_sb[:, h:h + 1], prod[:],
                                     axis=mybir.AxisListType.X)
            sm_sb = cpool.tile([128, SMALLW], f32, tag="smsb")
            nc.vector.tensor_copy(sm_sb[:], smp[:])
            nc.sync.dma_start(dg_o[:], diag_sb[:])
            nc.sync.dma_start(sm_o[:], sm_sb[:])

    _legalize_waits(nc, mybir)
    return nc


# ---------------------------------------------------------------------------
def _split_bf16(x):
    hi = x.astype(BF16)
    lo = (x - hi.astype(np.float32)).astype(BF16)
    return hi, lo


def _pack_dtiles(mat, nd):
    """[256or384, cols] -> [128, nd, cols] (partition-major d-tiles)."""
    d = mat.shape[0]
    if d < nd * 128:
        mat = np.concatenate(
            [mat, np.zeros((nd * 128 - d, mat.shape[1]), mat.dtype)], axis=0)
    return np.ascontiguousarray(
        mat.reshape(nd, 128, mat.shape[1]).transpose(1, 0, 2))


def host_prepare(embeddings, W_enc, b_enc, center_pool, edge_index,
                 joint_types, k):
    """Build adjacency-derived arrays + per-core rotated/packed in_maps."""
    ntot, d = embeddings.shape
    blk = ntot // NCORES
    G = ntot // 128
    HW_ = min(blk, 512)
    live = _live_blocks(ntot, blk)

    e0 = np.asarray(edge_index[0], np.int64) % ntot
    e1 = np.asarray(edge_index[1], np.int64) % ntot
    adj = np.zeros((ntot, ntot), dtype=np.uint8)
    adj[e0, e1] = 1
    adj |= adj.T  # symmetric 0/1, includes self-loops if present
    deg = adj.sum(axis=1, dtype=np.int64).astype(np.float32)
    dself = np.diagonal(adj).astype(np.float32)

    if TRI:
        # keep (i,j) once per unordered pair: d=(j-i) mod n in [1, n/2],
        # tie d==n/2 kept only for i<j; diagonal dropped (host adds it).
        jj = np.arange(ntot, dtype=np.int64)
        dmat = (jj[None, :] - jj[:, None]) % ntot
        keep = (dmat >= 1) & ((dmat < ntot // 2) |
                              ((dmat == ntot // 2) &
                               (jj[:, None] < jj[None, :])))
        a_use = (adj & keep).astype(FP8)
        del dmat, keep
    else:
        a_use = adj.astype(FP8)

    b = np.asarray(b_enc, np.float32)
    nd = 2 if not b.any() else 3
    embT = np.ascontiguousarray(embeddings.T.astype(np.float32))  # [256,ntot]
    wT = np.ascontiguousarray(W_enc.T.astype(np.float32))         # [256,256]
    if nd == 3:
        wT = np.concatenate([wT, b[None, :]], axis=0)             # [257,256]
    wt_hi, wt_lo = _split_bf16(_pack_dtiles(wT, nd))
    cen = _pack_dtiles(np.ascontiguousarray(
        center_pool[:64].T.astype(np.float32)), 2)
    ident = np.eye(128, dtype=np.float32)
    onehot = (np.asarray(joint_types)[:, None] ==
              np.arange(NTYPES)[None, :]).astype(np.float32)

    in_maps = []
    for c in range(NCORES):
        rot = (np.arange(ntot) + c * blk) % ntot
        # A block: rows rotated, cols = original cols [c*blk, c*blk+blk)
        a_rot = np.concatenate([a_use[c * blk:], a_use[:c * blk]], axis=0)
        a_blk = a_rot[:, c * blk:c * blk + blk]          # [ntot, blk]
        a4 = a_blk.reshape(G, 128, blk // HW_, HW_)
        # pack live blocks tile-major: [128, nlive*HW_]
        a_pk = np.empty((128, len(live) * HW_), dtype=FP8)
        for i, (g, h) in enumerate(live):
            a_pk[:, i * HW_:(i + 1) * HW_] = a4[g, :, h, :]
        eT = embT[:, rot]
        if nd == 3:
            eT = np.concatenate(
                [eT, np.ones((1, ntot), np.float32)], axis=0)
        ehi, elo = _split_bf16(_pack_dtiles(eT, nd))
        dgm = np.concatenate(
            [deg[rot][:, None], np.ones((ntot, 1), np.float32), onehot[rot]],
            axis=1)                                       # [ntot, 19]
        dgm = np.ascontiguousarray(
            dgm.reshape(G, 128, 2 + NTYPES).transpose(1, 0, 2)).astype(BF16)
        in_maps.append({
            "a_pk": np.ascontiguousarray(a_pk),
            "et_hi": np.ascontiguousarray(ehi),
            "et_lo": np.ascontiguousarray(elo),
            "wt_hi": np.ascontiguousarray(wt_hi),
            "wt_lo": np.ascontiguousarray(wt_lo),
            "cen": cen, "dgm": dgm, "ident": ident,
        })
    return in_maps, deg, dself, nd


def assemble(results, deg, dself, ntot, k, s_hint=None):
    """Gather per-core outputs into the reference's return tuple."""
    logits = np.concatenate([r["logits_sh"] for r in results], axis=0)
    s = np.concatenate([r["s_sh"] for r in results], axis=0)
    tr = np.float64(sum(float(r["diag"].sum()) for r in results))
    if TRI:
        selfrows = np.nonzero(dself > 0)[0]
        tr = 2.0 * tr + float((s[selfrows].astype(np.float64) ** 2).sum())
    sm = results[0]["small"].astype(np.float64)
    ss = sm[0:64, 0:64] + sm[0:64, 64:128] + sm[64:128, 0:64] + sm[64:128, 64:128]
    st_d = sm[0:64, 128] + sm[64:128, 128]
    csize = sm[0:64, 129] + sm[64:128, 129]
    typ = (sm[0:64, 130:SMALLW] + sm[64:128, 130:SMALLW]).T  # [17, 64]

    m = deg.astype(np.float64).sum() / 2.0
    if m < 1e-8:
        spectral = 0.0
    else:
        null = (st_d @ st_d) / (2.0 * m)
        spectral = -(tr - null) / (2.0 * m)
    ssn = ss / (np.linalg.norm(ss) + 1e-8)
    i_k = np.eye(64) / np.sqrt(float(k))
    ortho = np.linalg.norm(ssn - i_k)
    cluster = np.sqrt(float(k)) / ntot * np.linalg.norm(csize) - 1.0
    type_loss = (np.maximum(typ - 1.0, 0.0) ** 2).sum()
    return (logits, s, np.float32(spectral), np.float32(ortho),
            np.float32(cluster), np.float32(type_loss))


# ---------------------------------------------------------------------------
def run(inputs, trace=False, tmpdir=None):
    from concourse.bass_utils import run_bass_kernel_spmd

    emb = np.asarray(inputs["embeddings"], np.float32)
    kk = int(np.asarray(inputs["k"]))
    assert kk == 64, f"kernel specialized for k=64, got {kk}"
    ntot = emb.shape[0]
    blk = ntot // NCORES

    in_maps, deg, dself, nd = host_prepare(
        emb, inputs["W_enc"], inputs["b_enc"], inputs["center_pool"],
        inputs["edge_index"], inputs["joint_types"], kk)

    key = (ntot, blk, nd, TRI)
    if key not in _nc_cache:
        _nc_cache[key] = build_bass(ntot, blk, nd)
    nc = _nc_cache[key]

    br = run_bass_kernel_spmd(nc, in_maps, list(range(NCORES)),
                              trace=trace, tmpdir=tmpdir)
    outs = assemble(br.results, deg, dself, ntot, kk)
    return outs, br


def kernel(**inputs):
    outs, _ = run(inputs, trace=False)
    return outs


# revision 10
# speedup vs baseline: 2.8067x; 2.8067x over previous
"""DMoN head kernel for 8x Trainium2 NeuronCores (Bass/Tile, SPMD).

Strategy
--------
Only trace(S^T A S) is needed from the big adjacency contraction, and the
adjacency depends only on edge_index (an input), so the host builds the dense
0/1 adjacency (exact in fp8) and column-shards it over 8 cores.

Rotation trick: core c receives every row-indexed input rotated by c*BLK so
the single SPMD program uses only static slices -- each core's "first BLK
rows" are its own shard. Each core redundantly computes h = selu(emb@W^T+b),
logits, s = softmax (cheap), then contracts its A column-block:
  T = [s_hi|s_lo]^T @ A_blk   (bf16 hi/lo split of s packed as one 128-wide
                               stationary, fp8 A moving -> fp32-exact)
and reduces diag_k = sum_local s[local,k]*T[k,local] on-device.  The K-sized
side terms (ss = S^T S, S^T deg, cluster_size, M^T S) ride as ONE extra
matmul per contraction tile off the same stationary, into one PSUM bank.
Host sums the 8 per-core partials (the "KxK all-reduce" of the sharding
hint) and assembles the scalar losses.

TRI mode halves A traffic: keep each undirected pair once (wrap-distance
rule, core-independent under rotation), pack only live 512-wide blocks, and
let the host add the (tiny) self-loop term; trace = 2*sum(diag) + self_term.
"""

import os
import sys

import numpy as np

for _p in ("/opt/trn_rl_repo", "/root/.axon_site/_ro/trn_rl_repo"):
    if os.path.isdir(_p) and _p not in sys.path:
        sys.path.insert(0, _p)

import ml_dtypes  # noqa: E402

BF16 = ml_dtypes.bfloat16
FP8 = ml_dtypes.float8_e4m3fn

LAM = 1.0507009873554805  # selu lambda
ALPHA = 1.6732632423543772  # selu alpha
NCORES = 8
NTYPES = 17
HLW = 128 + 2 + NTYPES  # hilo(128) | deg | ones | one-hot(17)
SMALLW = HLW
TRI = os.environ.get("DMON_TRI", "1") == "1"

_nc_cache = {}


# ---------------------------------------------------------------------------
# static liveness for TRI mode: block of A_half rows r in [128g,128g+128),
# cols l in [512h, 512h+512) is live iff some (l-r) mod n lies in [1, n/2].
def _tri_live(ntot, g, h, hw):
    lo = (hw * h - 128 * g - 127) % ntot
    width = 128 + hw - 1
    for d in range(lo, lo + width + 1):
        dm = d % ntot
        if 1 <= dm <= ntot // 2:
            return True
    return False


def _live_blocks(ntot, blk):
    g_tiles = ntot // 128
    hw = min(blk, 512)
    nh = max(1, blk // hw)
    out = []
    for g in range(g_tiles):
        for h in range(nh):
            if not TRI or _tri_live(ntot, g, h, hw):
                out.append((g, h))
    return out


# ---------------------------------------------------------------------------
def _legalize_waits(nc, mybir):
    """This sandbox's walrus encodes at most ONE sync wait per instruction;
    Tile emits up to ~3. Hoist extra waits into standalone same-engine
    EventSemaphore (wait-only) instructions placed just before the owner."""
    n = 0
    for f in nc.m.functions:
        for blk in f.blocks:
            out = []
            for inst in blk.instructions:
                si = inst.sync_info
                if (si is not None and len(si.on_wait) > 1
                        and inst.opcode != "EventSemaphore"
                        and inst.engine != mybir.EngineType.Unassigned):
                    waits = list(si.on_wait)
                    for w in waits[:-1]:
                        n += 1
                        ev = mybir.InstEventSemaphore(
                            name=f"legw-{n}", engine=inst.engine,
                            sync_info=mybir.SyncInfo(on_wait=[w], on_update=[]))
                        nc.register_instruction(ev)
                        out.append(ev)
                    inst.sync_info = mybir.SyncInfo(
                        on_wait=[waits[-1]], on_update=list(si.on_update))
                out.append(inst)
            blk.instructions[:] = out
    return n


def build_bass(ntot, blk, nd=2):
    """Build the SPMD Bass program (identical on all cores).

    nd: number of 128-wide contraction tiles for the encoder matmul
        (2 for D=256; 3 when a nonzero b_enc is folded in as an extra
        ones-column contraction tile)."""
    import concourse.bass as bass
    import concourse.tile as tile
    from concourse import mybir

    f32 = mybir.dt.float32
    bf16 = mybir.dt.bfloat16
    f8e4 = mybir.dt.float8e4
    AF = mybir.ActivationFunctionType
    OP = mybir.AluOpType

    G = ntot // 128          # contraction tiles over rows
    NP = ntot // 512         # h/softmax panels (4 row-tiles each)
    HW_ = min(blk, 512)      # A column-block width
    NH = max(1, blk // HW_)  # A column blocks per core
    NOUT = blk // 128        # output row-tiles per core
    live = _live_blocks(ntot, blk)
    ncols_a = len(live) * HW_
    # A DMA chunking (tile-major packed [128, ncols_a] fp8)
    ACH = None
    for cand in (8192, 6144, 4096, 3072, 2048, 1536, 1024, 512, HW_):
        if cand <= ncols_a and ncols_a % cand == 0:
            ACH = cand
            break
    ECH = 1024 if ntot % 1024 == 0 else 512  # embT col chunk

    nc = bass.Bass()

    # register ln(alpha) as a const AP (activation float biases need one)
    lnALPHA = float(np.log(ALPHA))
    _cl = nc.alloc_sbuf_tensor("const-lnalpha", [128, 1], f32)
    nc.gpsimd.memset(_cl.ap(), lnALPHA)
    nc.const_aps.aps[(f32, lnALPHA)] = _cl.ap()
    nc.all_engine_barrier()

    # inputs (per core, pre-rotated+packed by host)
    A_d = nc.declare_dram_parameter("a_pk", [128, ncols_a], f8e4, isOutput=False)
    et_hi = nc.declare_dram_parameter("et_hi", [128, nd, ntot], bf16, isOutput=False)
    et_lo = nc.declare_dram_parameter("et_lo", [128, nd, ntot], bf16, isOutput=False)
    wt_hi = nc.declare_dram_parameter("wt_hi", [128, nd, 256], bf16, isOutput=False)
    wt_lo = nc.declare_dram_parameter("wt_lo", [128, nd, 256], bf16, isOutput=False)
    cen_d = nc.declare_dram_parameter("cen", [128, 2, 64], f32, isOutput=False)
    dgm_d = nc.declare_dram_parameter("dgm", [128, G, 2 + NTYPES], bf16,
                                      isOutput=False)
    id_d = nc.declare_dram_parameter("ident", [128, 128], f32, isOutput=False)

    # outputs
    lg_o = nc.declare_dram_parameter("logits_sh", [blk, 64], f32, isOutput=True)
    s_o = nc.declare_dram_parameter("s_sh", [blk, 64], f32, isOutput=True)
    dg_o = nc.declare_dram_parameter("diag", [64, NH], f32, isOutput=True)
    sm_o = nc.declare_dram_parameter("small", [128, SMALLW], f32, isOutput=True)

    with tile.TileContext(nc) as tc:
        with (
            tc.tile_pool(name="const", bufs=1) as cpool,
            tc.tile_pool(name="hT", bufs=1) as hpool,
            tc.tile_pool(name="achunk", bufs=3) as apool,
            tc.tile_pool(name="embc", bufs=2) as epool,
            tc.tile_pool(name="work", bufs=4) as wpool,
            tc.tile_pool(name="panel", bufs=3) as ppool,
            tc.tile_pool(name="tail", bufs=8) as tpool,
            tc.tile_pool(name="hps", bufs=2, space=bass.MemorySpace.PSUM) as hps,
            tc.tile_pool(name="lps", bufs=2, space=bass.MemorySpace.PSUM) as lps,
            tc.tile_pool(name="accps", bufs=1, space=bass.MemorySpace.PSUM) as accps,
        ):
            # hot-path constants first (gates the first matmul)
            wthi = cpool.tile([128, nd, 256], bf16, tag="wthi")
            wtlo = cpool.tile([128, nd, 256], bf16, tag="wtlo")
            nc.sync.dma_start(wthi[:], wt_hi[:])
            nc.sync.dma_start(wtlo[:], wt_lo[:])
            # first embT chunk
            et_hi_sb = epool.tile([128, nd, ECH], bf16, tag="ehi")
            et_lo_sb = epool.tile([128, nd, ECH], bf16, tag="elo")
            nc.sync.dma_start(et_hi_sb[:], et_hi[:, :, 0:ECH])
            nc.sync.dma_start(et_lo_sb[:], et_lo[:, :, 0:ECH])
            # remaining residents
            cen = cpool.tile([128, 2, 64], f32, tag="cen")
            dgm = cpool.tile([128, G, 2 + NTYPES], bf16, tag="dgm")
            idn = cpool.tile([128, 128], f32, tag="idn")
            nc.sync.dma_start(cen[:], cen_d[:])
            nc.sync.dma_start(dgm[:], dgm_d[:])
            nc.sync.dma_start(idn[:], id_d[:])

            sth = cpool.tile([64, blk], f32, tag="sth")
            diag_sb = cpool.tile([64, NH], f32, tag="diag")
            # persistent hT (f32) and manual-slot hilo+consts tile
            hT = [hpool.tile([128, ntot], f32, tag=f"hT{t}", name=f"hT{t}")
                  for t in range(2)]
            hl = cpool.tile([128, 8, HLW], bf16, tag="hl")

            # persistent PSUM accumulators
            TA = [accps.tile([128, HW_], f32, tag=f"TA{h}", name=f"TA{h}")
                  for h in range(NH)]
            smp = accps.tile([128, SMALLW], f32, tag="smp")

            a_sb = None
            a_next = 0  # next live-block index not yet covered by DMA
            li = 0      # live-block cursor
            last_g = {}
            first_g = {}
            for (g_, h_) in live:
                last_g[h_] = max(last_g.get(h_, -1), g_)
                first_g.setdefault(h_, g_)
            live_set = set(live)

            lnALPHA = float(np.log(ALPHA))

            for p in range(NP):
                if p > 0 and (p * 512) % ECH == 0:
                    c0 = p * 512
                    et_hi_sb = epool.tile([128, nd, ECH], bf16, tag="ehi")
                    et_lo_sb = epool.tile([128, nd, ECH], bf16, tag="elo")
                    nc.sync.dma_start(et_hi_sb[:], et_hi[:, :, c0:c0 + ECH])
                    nc.sync.dma_start(et_lo_sb[:], et_lo[:, :, c0:c0 + ECH])
                co = (p * 512) % ECH  # col offset within chunk
                sl = (p % 2) * 4      # hl slot range start for this panel

                # ---- h panel: hT[t1][:, p*512:(p+1)*512] = selu(x)/lam ----
                for t1 in range(2):
                    ph = hps.tile([128, 512], f32, tag="hps")
                    nmm = 0
                    for t0 in range(nd):
                        for (w, e) in ((wthi, et_hi_sb), (wtlo, et_hi_sb),
                                       (wthi, et_lo_sb)):
                            nc.tensor.matmul(
                                ph[:],
                                w[:, t0, t1 * 128:(t1 + 1) * 128],
                                e[:, t0, co:co + 512],
                                start=(nmm == 0), stop=(nmm == 3 * nd - 1),
                            )
                            nmm += 1
                    # selu(x)/lam = relu(x) + (min(alpha*e^x, alpha) - alpha)
                    ex = wpool.tile([128, 512], f32, tag="ex")
                    nc.scalar.activation(ex[:], ph[:], AF.Exp, bias=lnALPHA,
                                         scale=1.0)
                    rl = wpool.tile([128, 512], f32, tag="rl")
                    nc.scalar.activation(rl[:], ph[:], AF.Relu)
                    mn = wpool.tile([128, 512], f32, tag="mn")
                    nc.vector.tensor_scalar(mn[:], ex[:], ALPHA, -ALPHA,
                                            OP.min, OP.add)
                    nc.vector.tensor_add(hT[t1][:, p * 512:(p + 1) * 512],
                                         mn[:], rl[:])

                # per-g side constants into hl slot tails (one op per panel)
                nc.vector.tensor_copy(hl[:, sl:sl + 4, 128:HLW],
                                      dgm[:, p * 4:p * 4 + 4, :])

                # ---- logits for the 4 row-tiles of this panel ----
                lgp = ppool.tile([128, 4, 64], f32, tag="lgp")
                for gi in range(4):
                    g = p * 4 + gi
                    lp = lps.tile([128, 64], f32, tag="lg")
                    nc.tensor.matmul(lp[:],
                                     hT[0][:, g * 128:(g + 1) * 128],
                                     cen[:, 0, :], start=True, stop=False)
                    nc.tensor.matmul(lp[:],
                                     hT[1][:, g * 128:(g + 1) * 128],
                                     cen[:, 1, :], start=False, stop=True)
                    nc.scalar.activation(lgp[:, gi, :], lp[:], AF.Copy,
                                         bias=0.0, scale=LAM / 16.0)

                # ---- panel softmax (no max-sub; |logits| is O(1)) ----
                ep = ppool.tile([128, 4, 64], f32, tag="ep")
                nc.scalar.activation(ep[:], lgp[:], AF.Exp)
                rs = ppool.tile([128, 4], f32, tag="rs")
                nc.vector.tensor_reduce(rs[:], ep[:], mybir.AxisListType.X,
                                        OP.add)
                rc = ppool.tile([128, 4, 1], f32, tag="rc")
                nc.vector.reciprocal(rc[:, :, 0], rs[:])
                sp = ppool.tile([128, 4, 64], f32, tag="sp")
                nc.vector.tensor_tensor(sp[:], ep[:],
                                        rc[:].broadcast_to([128, 4, 64]),
                                        op=OP.mult)
                # bf16 hi/lo split into hl slots
                nc.vector.tensor_copy(hl[:, sl:sl + 4, 0:64], sp[:])
                nc.vector.tensor_sub(hl[:, sl:sl + 4, 64:128], sp[:],
                                     hl[:, sl:sl + 4, 0:64])

                # ---- per row-tile: outputs, transpose, A-chain ----
                for gi in range(4):
                    g = p * 4 + gi
                    s = sl + gi
                    if g < NOUT:
                        nc.sync.dma_start(lg_o[g * 128:(g + 1) * 128, :],
                                          lgp[:, gi, :])
                        nc.sync.dma_start(s_o[g * 128:(g + 1) * 128, :],
                                          sp[:, gi, :])
                        tp = lps.tile([64, 128], f32, tag="lg")
                        nc.tensor.transpose(tp[:], sp[:, gi, :], idn[:])
                        nc.vector.tensor_copy(sth[:, g * 128:(g + 1) * 128],
                                              tp[:])
                    for h in range(NH):
                        if (g, h) not in live_set:
                            continue
                        if li >= a_next:  # need next A chunk
                            a_sb = apool.tile([128, ACH], f8e4, tag="ach")
                            nc.sync.dma_start(
                                a_sb[:],
                                A_d[:, a_next * HW_:a_next * HW_ + ACH])
                            a_next += ACH // HW_
                        base = (li % (ACH // HW_)) * HW_
                        nc.tensor.matmul(TA[h][:], hl[:, s, 0:128],
                                         a_sb[:, base:base + HW_],
                                         start=(g == first_g[h]),
                                         stop=(g == last_g[h]),
                                         skip_group_check=True)
                        li += 1
                    # all K-sized side terms in one matmul
                    nc.tensor.matmul(smp[:], hl[:, s, 0:128], hl[:, s, :],
                                     start=(g == 0), stop=(g == G - 1),
                                     skip_group_check=True)

            # ---- tail: T = hi+lo, diag_k = sum_l T*s^T, copy small out ----
            for h in range(NH):
                lo_sb = tpool.tile([64, HW_], f32, tag="tl")
                nc.vector.tensor_copy(lo_sb[:], TA[h][64:128, :])
                t_sb = tpool.tile([64, HW_], f32, tag="tl")
                nc.vector.tensor_add(t_sb[:], TA[h][0:64, :], lo_sb[:])
                prod = tpool.tile([64, HW_], f32, tag="tl")
                nc.vector.tensor_mul(prod[:], t_sb[:],
                                     sth[:, h * HW_:(h + 1) * HW_])
                nc.vector.reduce_sum(diag_sb[:, h:h + 1], prod[:],
                                     axis=mybir.AxisListType.X)
            sm_sb = cpool.tile([128, SMALLW], f32, tag="smsb")
            nc.vector.tensor_copy(sm_sb[:], smp[:])
            nc.sync.dma_start(dg_o[:], diag_sb[:])
            nc.sync.dma_start(sm_o[:], sm_sb[:])

    _legalize_waits(nc, mybir)
    return nc


# ---------------------------------------------------------------------------
def _split_bf16(x):
    hi = x.astype(BF16)
    lo = (x - hi.astype(np.float32)).astype(BF16)
    return hi, lo


def _pack_dtiles(mat, nd):
    """[256or384, cols] -> [128, nd, cols] (partition-major d-tiles)."""
    d = mat.shape[0]
    if d < nd * 128:
        mat = np.concatenate(
            [mat, np.zeros((nd * 128 - d, mat.shape[1]), mat.dtype)], axis=0)
    return np.ascontiguousarray(
        mat.reshape(nd, 128, mat.shape[1]).transpose(1, 0, 2))


def host_prepare(embeddings, W_enc, b_enc, center_pool, edge_index,
                 joint_types, k):
    """Build adjacency-derived arrays + per-core rotated/packed in_maps."""
    ntot, d = embeddings.shape
    blk = ntot // NCORES
    G = ntot // 128
    HW_ = min(blk, 512)
    live = _live_blocks(ntot, blk)

    e0 = np.asarray(edge_index[0], np.int64) % ntot
    e1 = np.asarray(edge_index[1], np.int64) % ntot
    adj = np.zeros((ntot, ntot), dtype=np.uint8)
    adj[e0, e1] = 1
    adj |= adj.T  # symmetric 0/1, includes self-loops if present
    deg = adj.sum(axis=1, dtype=np.int64).astype(np.float32)
    dself = np.diagonal(adj).astype(np.float32)

    if TRI:
        # keep (i,j) once per unordered pair: d=(j-i) mod n in [1, n/2],
        # tie d==n/2 kept only for i<j; diagonal dropped (host adds it).
        jj = np.arange(ntot, dtype=np.int64)
        dmat = (jj[None, :] - jj[:, None]) % ntot
        keep = (dmat >= 1) & ((dmat < ntot // 2) |
                              ((dmat == ntot // 2) &
                               (jj[:, None] < jj[None, :])))
        a_use = (adj & keep).astype(FP8)
        del dmat, keep
    else:
        a_use = adj.astype(FP8)

    b = np.asarray(b_enc, np.float32)
    nd = 2 if not b.any() else 3
    embT = np.ascontiguousarray(embeddings.T.astype(np.float32))  # [256,ntot]
    wT = np.ascontiguousarray(W_enc.T.astype(np.float32))         # [256,256]
    if nd == 3:
        wT = np.concatenate([wT, b[None, :]], axis=0)             # [257,256]
    wt_hi, wt_lo = _split_bf16(_pack_dtiles(wT, nd))
    cen = _pack_dtiles(np.ascontiguousarray(
        center_pool[:64].T.astype(np.float32)), 2)
    ident = np.eye(128, dtype=np.float32)
    onehot = (np.asarray(joint_types)[:, None] ==
              np.arange(NTYPES)[None, :]).astype(np.float32)

    in_maps = []
    for c in range(NCORES):
        rot = (np.arange(ntot) + c * blk) % ntot
        # A block: rows rotated, cols = original cols [c*blk, c*blk+blk)
        a_rot = np.concatenate([a_use[c * blk:], a_use[:c * blk]], axis=0)
        a_blk = a_rot[:, c * blk:c * blk + blk]          # [ntot, blk]
        a4 = a_blk.reshape(G, 128, blk // HW_, HW_)
        # pack live blocks tile-major: [128, nlive*HW_]
        a_pk = np.empty((128, len(live) * HW_), dtype=FP8)
        for i, (g, h) in enumerate(live):
            a_pk[:, i * HW_:(i + 1) * HW_] = a4[g, :, h, :]
        eT = embT[:, rot]
        if nd == 3:
            eT = np.concatenate(
                [eT, np.ones((1, ntot), np.float32)], axis=0)
        ehi, elo = _split_bf16(_pack_dtiles(eT, nd))
        dgm = np.concatenate(
            [deg[rot][:, None], np.ones((ntot, 1), np.float32), onehot[rot]],
            axis=1)                                       # [ntot, 19]
        dgm = np.ascontiguousarray(
            dgm.reshape(G, 128, 2 + NTYPES).transpose(1, 0, 2)).astype(BF16)
        in_maps.append({
            "a_pk": np.ascontiguousarray(a_pk),
            "et_hi": np.ascontiguousarray(ehi),
            "et_lo": np.ascontiguousarray(elo),
            "wt_hi": np.ascontiguousarray(wt_hi),
            "wt_lo": np.ascontiguousarray(wt_lo),
            "cen": cen, "dgm": dgm, "ident": ident,
        })
    return in_maps, deg, dself, nd


def assemble(results, deg, dself, ntot, k, s_hint=None):
    """Gather per-core outputs into the reference's return tuple."""
    logits = np.concatenate([r["logits_sh"] for r in results], axis=0)
    s = np.concatenate([r["s_sh"] for r in results], axis=0)
    tr = np.float64(sum(float(r["diag"].sum()) for r in results))
    if TRI:
        selfrows = np.nonzero(dself > 0)[0]
        tr = 2.0 * tr + float((s[selfrows].astype(np.float64) ** 2).sum())
    sm = results[0]["small"].astype(np.float64)
    ss = sm[0:64, 0:64] + sm[0:64, 64:128] + sm[64:128, 0:64] + sm[64:128, 64:128]
    st_d = sm[0:64, 128] + sm[64:128, 128]
    csize = sm[0:64, 129] + sm[64:128, 129]
    typ = (sm[0:64, 130:SMALLW] + sm[64:128, 130:SMALLW]).T  # [17, 64]

    m = deg.astype(np.float64).sum() / 2.0
    if m < 1e-8:
        spectral = 0.0
    else:
        null = (st_d @ st_d) / (2.0 * m)
        spectral = -(tr - null) / (2.0 * m)
    ssn = ss / (np.linalg.norm(ss) + 1e-8)
    i_k = np.eye(64) / np.sqrt(float(k))
    ortho = np.linalg.norm(ssn - i_k)
    cluster = np.sqrt(float(k)) / ntot * np.linalg.norm(csize) - 1.0
    type_loss = (np.maximum(typ - 1.0, 0.0) ** 2).sum()
    return (logits, s, np.float32(spectral), np.float32(ortho),
            np.float32(cluster), np.float32(type_loss))


# ---------------------------------------------------------------------------
def run(inputs, trace=False, tmpdir=None):
    from concourse.bass_utils import run_bass_kernel_spmd

    emb = np.asarray(inputs["embeddings"], np.float32)
    kk = int(np.asarray(inputs["k"]))
    assert kk == 64, f"kernel specialized for k=64, got {kk}"
    ntot = emb.shape[0]
    blk = ntot // NCORES

    in_maps, deg, dself, nd = host_prepare(
        emb, inputs["W_enc"], inputs["b_enc"], inputs["center_pool"],
        inputs["edge_index"], inputs["joint_types"], kk)

    key = (ntot, blk, nd, TRI)
    if key not in _nc_cache:
        _nc_cache[key] = build_bass(ntot, blk, nd)
    nc = _nc_cache[key]

    br = run_bass_kernel_spmd(nc, in_maps, list(range(NCORES)),
                              trace=trace, tmpdir=tmpdir)
    outs = assemble(br.results, deg, dself, ntot, kk)
    return outs, br


def kernel(**inputs):
    outs, _ = run(inputs, trace=False)
    return outs
